# revision 1
# baseline (speedup 1.0000x reference)
"""Cross-attention block (nn_CABlock) on 8 TRN2 NeuronCores.

Reference (per batch b):
    q  = xq @ Wq.T            -> [SQ, H]   split heads [SQ, 16, 64]
    kv = xkv @ Wkv.T          -> [SKV, 2H] split [SKV, 2, 16, 64]
    att = softmax(q k^T / sqrt(64))
    x   = att @ v  (merge heads)
    out = x @ Wout.T + bout

Sharding: 8 cores = 4 batches x 2 head-groups (8 heads each).  Each core
computes its batch's projections restricted to its 8 heads, attention for
those heads, and a partial out-projection (contraction over its 512
hd-columns of Wout).  Host sums the two partials per batch and adds bout.

All matmuls run in fp32r (fp32 storage, reduced-precision PE mode at bf16
speed, ~1.5e-4 matmul rel err).  Per-core layout:
  - Host pre-transposes activations: xqT [H, SQ], xkvT [H, SKV].
  - q/k projections produce qT/kT with hd on partitions in natural head
    order (head = hd//64, so head h lives in chunk h//2 at partition
    offset (h%2)*64 -- all partition offsets stay 32-aligned).
  - v projection produces v_aug [SKV, 8 heads x 65] with a ones column
    per head: att @ v_aug then also emits the softmax denominator Z as
    row 64.  No max subtraction (scores are O(1), exp cannot overflow).
  - scores are computed transposed, sT[y, x], two 128-y tiles into one
    2-bank PSUM tile; a single W=1024 Exp on the scalar engine (fuses the
    1/8 scale) amortizes the ~250ns ACT fixed cost; fp32r attention
    weights feed att @ v_aug accumulation into psum [65, 512].
  - xTu [512, SQ] collects unnormalized head outputs (natural hd order);
    Z rows are replicated to 128 partitions with K=1 ones-matmuls,
    reciprocal'd on DVE, and multiplied in place once a (2j, 2j+1) head
    pair completes.
  - out-projection contracts the 4 xTu chunks with WoutT slices (host
    pre-transposed, natural order), streaming [128, 512] tiles to DRAM.
"""

import sys

sys.path.insert(0, "/opt/trn_rl_repo")

import numpy as np

import concourse.bass as bass
import concourse.mybir as mybir
import concourse.tile as tile
from concourse.bass_utils import run_bass_kernel_spmd
from concourse.tile import add_dep_helper

F32 = mybir.dt.float32
F32R = mybir.dt.float32r

HIDDEN = 1024
NUM_HEADS = 16
HEAD_DIM = 64
B = 4
SQ = 1024
SKV = 2048
NCORES = 8
NHL = 8          # heads per core
HL = NHL * HEAD_DIM  # 512, local hd width
SCALE = HEAD_DIM ** -0.5
KCH = HIDDEN // 128  # 8 contraction chunks for the projections
NYC = SKV // 128     # 16 key/value row chunks


def _legalize_waits(nc, limit=1):
    """The walrus build in this container accepts only ~1 sync-wait per
    instruction struct; spill excess waits onto preceding engine NoOps."""
    import bass_rust

    ctr = 0
    for fn in nc.m.functions:
        for blk in fn.blocks:
            out = []
            changed = False
            for inst in blk.instructions:
                si = inst.sync_info
                ws = list(si.on_wait) if si is not None and si.on_wait else []
                if len(ws) > limit:
                    spill, keep = ws[:-limit], ws[-limit:]
                    for w in spill:
                        ctr += 1
                        nop = mybir.InstNoOp(name=f"ant-waitnop-{ctr}", ins=[], outs=[])
                        nop.engine = inst.engine
                        nop.sync_info = bass_rust.SyncInfo(on_wait=[w], on_update=[])
                        out.append(nop)
                    si.on_wait = keep
                    changed = True
                out.append(inst)
            if changed:
                blk.instructions = out
    return ctr


def _emit(nc, tc):
    xqT = nc.dram_tensor("xqT", [HIDDEN, SQ], F32R, kind="ExternalInput")
    xkvT = nc.dram_tensor("xkvT", [HIDDEN, SKV], F32R, kind="ExternalInput")
    WqT = nc.dram_tensor("WqT", [HIDDEN, HL], F32R, kind="ExternalInput")
    WkT = nc.dram_tensor("WkT", [HIDDEN, HL], F32R, kind="ExternalInput")
    WvT = nc.dram_tensor("WvT", [HIDDEN, HL], F32R, kind="ExternalInput")
    WoT = nc.dram_tensor("WoT", [HL, HIDDEN], F32R, kind="ExternalInput")
    onesA = nc.dram_tensor("onesA", [1, 128], F32R, kind="ExternalInput")
    onesB = nc.dram_tensor("onesB", [1, 128], F32R, kind="ExternalInput")
    out_d = nc.dram_tensor("out", [SQ, HIDDEN], F32, kind="ExternalOutput")

    with tc.tile_pool(name="persist", bufs=1) as pp:
        qT = pp.tile([128, 4, SQ], F32R)          # 16 KB/part
        onesA_t = pp.tile([1, 128], F32R)
        onesB_t = pp.tile([1, 128], F32R)
        kT = pp.tile([128, 4, SKV], F32R)         # 32 KB
        va = pp.tile([128, NYC, NHL, 65], F32R)   # 33.3 KB  (y-chunk, head, dv+1)
        xTu = pp.tile([128, 4, SQ], F32R)         # 16 KB, natural hd order
        nc.sync.dma_start(out=onesA_t, in_=onesA[:, :])
        nc.sync.dma_start(out=onesB_t, in_=onesB[:, :])

        with tc.tile_pool(name="psProj", bufs=4, space="PSUM") as psProj:
            # ---- Phase 1: q projection  qT[hd, s] = sum_i Wq[hd, i] xq[s, i]
            with tc.tile_pool(name="qin", bufs=1) as qin:
                xqT_t = qin.tile([128, KCH, SQ], F32R)
                WqT_t = qin.tile([128, KCH, HL], F32R)
                nc.sync.dma_start(
                    out=WqT_t, in_=WqT.rearrange("(k p) n -> p k n", p=128)
                )
                for k in range(KCH):
                    nc.sync.dma_start(
                        out=xqT_t[:, k, :], in_=xqT[k * 128 : (k + 1) * 128, :]
                    )
                # k outermost: compute starts as soon as the first chunks land
                for s in range(2):
                    pts = [psProj.tile([128, 512], F32, tag="proj", name="ptq") for _ in range(4)]
                    for k in range(KCH):
                        for m in range(4):
                            nc.tensor.matmul(
                                pts[m][:, :],
                                lhsT=WqT_t[:, k, m * 128 : (m + 1) * 128],
                                rhs=xqT_t[:, k, s * 512 : (s + 1) * 512],
                                start=(k == 0),
                                stop=(k == KCH - 1),
                            )
                    for m in range(4):
                        nc.vector.tensor_copy(
                            out=qT[:, m, s * 512 : (s + 1) * 512], in_=pts[m][:, :]
                        )

            # ---- Phase 2: k and v projections from xkvT
            with tc.tile_pool(name="kvx", bufs=1) as kvx:
                xkvT_t = kvx.tile([128, KCH, SKV], F32R)
                for k in range(KCH):
                    nc.sync.dma_start(
                        out=xkvT_t[:, k, :], in_=xkvT[k * 128 : (k + 1) * 128, :]
                    )
                with tc.tile_pool(name="kvw1", bufs=1) as kvw1:
                    WkT_t = kvw1.tile([128, KCH, HL], F32R)
                    nc.sync.dma_start(
                        out=WkT_t, in_=WkT.rearrange("(k p) n -> p k n", p=128)
                    )
                    # kT[hd, y], k outermost within each y-group
                    for yg in range(SKV // 512):
                        pts = [
                            psProj.tile([128, 512], F32, tag="proj", name="ptk")
                            for _ in range(4)
                        ]
                        for k in range(KCH):
                            for m in range(4):
                                nc.tensor.matmul(
                                    pts[m][:, :],
                                    lhsT=WkT_t[:, k, m * 128 : (m + 1) * 128],
                                    rhs=xkvT_t[:, k, yg * 512 : (yg + 1) * 512],
                                    start=(k == 0),
                                    stop=(k == KCH - 1),
                                )
                        for m in range(4):
                            nc.vector.tensor_copy(
                                out=kT[:, m, yg * 512 : (yg + 1) * 512],
                                in_=pts[m][:, :],
                            )
                with tc.tile_pool(name="kvw2", bufs=1) as kvw2:
                    WvT_t = kvw2.tile([128, KCH, HL], F32R)
                    nc.sync.dma_start(
                        out=WvT_t, in_=WvT.rearrange("(k p) n -> p k n", p=128)
                    )
                    # v_aug[y, h, 0:64] in two half-width passes (heads 0-3,
                    # then 4-7) so early attention chains unblock sooner
                    for half in range(2):
                        hlo = half * 4
                        for yc in range(NYC):
                            pt = psProj.tile([128, 512], F32, tag="proj")
                            for k in range(KCH):
                                nc.tensor.matmul(
                                    pt[:, 0:256],
                                    lhsT=xkvT_t[:, k, yc * 128 : (yc + 1) * 128],
                                    rhs=WvT_t[:, k, hlo * 64 : (hlo + 4) * 64],
                                    start=(k == 0),
                                    stop=(k == KCH - 1),
                                )
                            nc.vector.tensor_copy(
                                out=va[:, yc, hlo : hlo + 4, 0:64],
                                in_=pt[:, 0:256].rearrange("p (h d) -> p h d", h=4),
                            )
                    nc.vector.memset(va[:, :, :, 64:65].bitcast(F32), 1.0)

        # ---- Phase 3: attention + normalization + out projection
        with (
            tc.tile_pool(name="attp", bufs=1) as attp,
            tc.tile_pool(name="zrowp", bufs=4) as zrowp,
            tc.tile_pool(name="rzp", bufs=2) as rzp,
            tc.tile_pool(name="outw", bufs=1) as outw,
            tc.tile_pool(name="outstage", bufs=3) as outstage,
            tc.tile_pool(name="psS2", bufs=1, space="PSUM") as psS2,
            tc.tile_pool(name="psMisc", bufs=1, space="PSUM") as psMisc,
        ):
            WoT_t = outw.tile([128, 4, HIDDEN], F32R)
            nc.sync.dma_start(
                out=WoT_t, in_=WoT.rearrange("(j p) n -> p j n", p=128)
            )

            NYB = NYC // 2  # 8 double-y blocks per chain
            LAG = 3         # attv trails scores/exp by this many blocks

            _scnt = [0]

            def emit_scores(h, xc, yb):
                pq = (h % 2) * 64
                m = h // 2
                _scnt[0] += 1
                pscr = psS2.tile(
                    [128, 2, 512], F32, tag=f"scores{_scnt[0] % 3}", name="pscr"
                )
                last = None
                for i in range(2):
                    yc = 2 * yb + i
                    last = nc.tensor.matmul(
                        pscr[:, i, :],
                        lhsT=kT[pq : pq + 64, m, yc * 128 : (yc + 1) * 128],
                        rhs=qT[pq : pq + 64, m, xc * 512 : (xc + 1) * 512],
                        start=True,
                        stop=True,
                    )
                at = attp.tile(
                    [128, 2, 512], F32R, tag=f"att{_scnt[0] % 10}", name="at"
                )
                nc.scalar.activation(
                    out=at[:, :, :].rearrange("p a b -> p (a b)"),
                    in_=pscr[:, :, :].rearrange("p a b -> p (a b)"),
                    func=mybir.ActivationFunctionType.Exp,
                    scale=SCALE,
                )
                return at, last

            def emit_attv(h, po, at, yb, order_after=None):
                for i in range(2):
                    yc = 2 * yb + i
                    mm = nc.tensor.matmul(
                        po[:, :],
                        lhsT=va[:, yc, h, :],
                        rhs=at[:, i, :],
                        start=(yc == 0),
                        stop=(yc == NYC - 1),
                    )
                    if i == 0 and order_after is not None:
                        # force the PE static order to keep attv trailing the
                        # scores stream by LAG blocks (hides the ACT latency)
                        add_dep_helper(
                            mm.ins,
                            order_after.ins,
                            sync=False,
                            reason="attv trails scores pipeline",
                        )

            for xc in range(SQ // 512):
                for j in range(4):
                    hA, hB = 2 * j, 2 * j + 1
                    poA = psMisc.tile([65, 512], F32, tag="attv", bufs=1, name="poA")
                    poB = psMisc.tile([65, 512], F32, tag="attv2", bufs=1, name="poB")
                    attsA, attsB = [], []
                    # two independent chains interleaved: while one waits on
                    # ACT, the PE works the other
                    scoreMM = []
                    for yb in range(NYB):
                        atA, _ = emit_scores(hA, xc, yb)
                        atB, lastB = emit_scores(hB, xc, yb)
                        attsA.append(atA)
                        attsB.append(atB)
                        scoreMM.append(lastB)
                        if yb >= LAG:
                            emit_attv(hA, poA, attsA[yb - LAG], yb - LAG,
                                      order_after=scoreMM[yb])
                            emit_attv(hB, poB, attsB[yb - LAG], yb - LAG)
                    for yb in range(NYB - LAG, NYB):
                        emit_attv(hA, poA, attsA[yb], yb)
                        emit_attv(hB, poB, attsB[yb], yb)

                    zrs = []
                    for h, po in ((hA, poA), (hB, poB)):
                        ps_off = (h % 2) * 64
                        nc.vector.tensor_copy(
                            out=xTu[
                                ps_off : ps_off + 64, j, xc * 512 : (xc + 1) * 512
                            ],
                            in_=po[0:64, :],
                        )
                        zr = zrowp.tile([1, 512], F32R, tag="zrow", name="zr")
                        nc.vector.tensor_copy(out=zr[0:1, :], in_=po[64:65, :])
                        zrs.append(zr)
                    # normalize chunk j (bcast psum shares the scores slots)
                    pb = psS2.tile([128, 512], F32, tag="scores0", name="pb")
                    nc.tensor.matmul(
                        pb[:, :], lhsT=onesA_t[0:1, :], rhs=zrs[0][0:1, :],
                        start=True, stop=False,
                    )
                    nc.tensor.matmul(
                        pb[:, :], lhsT=onesB_t[0:1, :], rhs=zrs[1][0:1, :],
                        start=False, stop=True,
                    )
                    rz = rzp.tile([128, 512], F32, tag="rz", name="rz")
                    nc.vector.reciprocal(out=rz[:, :], in_=pb[:, :])
                    nc.vector.tensor_mul(
                        xTu[:, j, xc * 512 : (xc + 1) * 512],
                        xTu[:, j, xc * 512 : (xc + 1) * 512],
                        rz[:, :],
                    )

            # out projection: out[s, o] = sum_j xTu[:, j, s].T @ WoT[:, j, o]
            for sc in range(SQ // 128):
                for oc in range(HIDDEN // 512):
                    pt = psS2.tile([128, 512], F32, tag="scores1", name="pto")
                    for j in range(4):
                        nc.tensor.matmul(
                            pt[:, :],
                            lhsT=xTu[:, j, sc * 128 : (sc + 1) * 128],
                            rhs=WoT_t[:, j, oc * 512 : (oc + 1) * 512],
                            start=(j == 0),
                            stop=(j == 3),
                        )
                    ot = outstage.tile([128, 512], F32, tag="out")
                    nc.vector.tensor_copy(out=ot[:, :], in_=pt[:, :])
                    nc.sync.dma_start(
                        out=out_d[sc * 128 : (sc + 1) * 128, oc * 512 : (oc + 1) * 512],
                        in_=ot[:, :],
                    )


_NC = None


def _get_nc():
    global _NC
    if _NC is None:
        nc = bass.Bass(trn_type="TRN2")
        with tile.TileContext(nc) as tc:
            _emit(nc, tc)
        _legalize_waits(nc)
        _NC = nc
    return _NC


def _prep_inputs(xq, xkv, Wq, Wkv, Wout):
    xq = np.asarray(xq, dtype=np.float32)
    xkv = np.asarray(xkv, dtype=np.float32)
    Wq = np.asarray(Wq, dtype=np.float32)
    Wkv = np.asarray(Wkv, dtype=np.float32)
    Wout = np.asarray(Wout, dtype=np.float32)

    onesA = np.zeros((1, 128), np.float32)
    onesA[0, 0:64] = 1.0
    onesB = np.zeros((1, 128), np.float32)
    onesB[0, 64:128] = 1.0

    xqT = [np.ascontiguousarray(xq[b].T) for b in range(B)]
    xkvT = [np.ascontiguousarray(xkv[b].T) for b in range(B)]

    per_hg = []
    for hg in range(2):
        hs = slice(hg * HL, (hg + 1) * HL)
        WqTh = np.ascontiguousarray(Wq[hs].T)
        WkTh = np.ascontiguousarray(Wkv[hs].T)
        WvTh = np.ascontiguousarray(Wkv[HIDDEN + hg * HL : HIDDEN + (hg + 1) * HL].T)
        WoTh = np.ascontiguousarray(Wout[:, hs].T)
        per_hg.append((WqTh, WkTh, WvTh, WoTh))

    in_maps = []
    for c in range(NCORES):
        b, hg = c // 2, c % 2
        WqTh, WkTh, WvTh, WoTh = per_hg[hg]
        in_maps.append(
            {
                "xqT": xqT[b],
                "xkvT": xkvT[b],
                "WqT": WqTh,
                "WkT": WkTh,
                "WvT": WvTh,
                "WoT": WoTh,
                "onesA": onesA,
                "onesB": onesB,
            }
        )
    return in_maps


def run_sharded(xq, xkv, Wq, Wkv, Wout, bout, trace=False, **kwargs):
    """Build+run the SPMD kernel; returns (full_output, BassKernelResults)."""
    nc = _get_nc()
    in_maps = _prep_inputs(xq, xkv, Wq, Wkv, Wout)
    res = run_bass_kernel_spmd(
        nc, in_maps, core_ids=list(range(NCORES)), trace=trace, **kwargs
    )
    bout = np.asarray(bout, dtype=np.float32)
    out = np.empty((B, SQ, HIDDEN), np.float32)
    for b in range(B):
        out[b] = res.results[2 * b]["out"] + res.results[2 * b + 1]["out"]
    out += bout[None, None, :]
    return out, res


def kernel(xq, xkv, Wq, Wkv, Wout, bout):
    out, _ = run_sharded(xq, xkv, Wq, Wkv, Wout, bout)
    return out



# revision 2
# speedup vs baseline: 1.1720x; 1.1720x over previous
"""Cross-attention block (nn_CABlock) on 8 TRN2 NeuronCores.

Reference (per batch b):
    q  = xq @ Wq.T            -> [SQ, H]   split heads [SQ, 16, 64]
    kv = xkv @ Wkv.T          -> [SKV, 2H] split [SKV, 2, 16, 64]
    att = softmax(q k^T / sqrt(64))
    x   = att @ v  (merge heads)
    out = x @ Wout.T + bout

Sharding: 8 cores = 4 batches x 2 head-groups (8 heads each).  Each core
computes its batch's projections restricted to its 8 heads, attention for
those heads, and a partial out-projection (contraction over its 512
hd-columns of Wout).  Host sums the two partials per batch and adds bout.

All matmuls run in fp32r (fp32 storage, reduced-precision PE mode at bf16
speed, ~1.5e-4 matmul rel err).  Per-core layout:
  - Host pre-transposes activations: xqT [H, SQ], xkvT [H, SKV].
  - q/k projections produce qT/kT with hd on partitions in natural head
    order (head = hd//64, so head h lives in chunk h//2 at partition
    offset (h%2)*64 -- all partition offsets stay 32-aligned).
  - v projection produces v_aug [SKV, 8 heads x 65] with a ones column
    per head: att @ v_aug then also emits the softmax denominator Z as
    row 64.  No max subtraction (scores are O(1), exp cannot overflow).
  - scores are computed transposed, sT[y, x], two 128-y tiles into one
    2-bank PSUM tile; a single W=1024 Exp on the scalar engine (fuses the
    1/8 scale) amortizes the ~250ns ACT fixed cost; fp32r attention
    weights feed att @ v_aug accumulation into psum [65, 512].
  - xTu [512, SQ] collects unnormalized head outputs (natural hd order);
    Z rows are replicated to 128 partitions with K=1 ones-matmuls,
    reciprocal'd on DVE, and multiplied in place once a (2j, 2j+1) head
    pair completes.
  - out-projection contracts the 4 xTu chunks with WoutT slices (host
    pre-transposed, natural order), streaming [128, 512] tiles to DRAM.
"""

import sys

sys.path.insert(0, "/opt/trn_rl_repo")

import numpy as np

import concourse.bass as bass
import concourse.mybir as mybir
import concourse.tile as tile
from concourse.bass_utils import run_bass_kernel_spmd
from concourse.tile import add_dep_helper

F32 = mybir.dt.float32
F32R = mybir.dt.float32r
F16 = mybir.dt.float16

HIDDEN = 1024
NUM_HEADS = 16
HEAD_DIM = 64
B = 4
SQ = 1024
SKV = 2048
NCORES = 8
NHL = 8          # heads per core
HL = NHL * HEAD_DIM  # 512, local hd width
SCALE = HEAD_DIM ** -0.5
KCH = HIDDEN // 128  # 8 contraction chunks for the projections
NYC = SKV // 128     # 16 key/value row chunks


def _legalize_waits(nc, limit=1):
    """The walrus build in this container accepts only ~1 sync-wait per
    instruction struct; spill excess waits onto preceding engine NoOps."""
    import bass_rust

    ctr = 0
    for fn in nc.m.functions:
        for blk in fn.blocks:
            out = []
            changed = False
            for inst in blk.instructions:
                si = inst.sync_info
                ws = list(si.on_wait) if si is not None and si.on_wait else []
                if len(ws) > limit:
                    spill, keep = ws[:-limit], ws[-limit:]
                    for w in spill:
                        ctr += 1
                        nop = mybir.InstNoOp(name=f"ant-waitnop-{ctr}", ins=[], outs=[])
                        nop.engine = inst.engine
                        nop.sync_info = bass_rust.SyncInfo(on_wait=[w], on_update=[])
                        out.append(nop)
                    si.on_wait = keep
                    changed = True
                out.append(inst)
            if changed:
                blk.instructions = out
    return ctr


def _emit(nc, tc):
    xqT = nc.dram_tensor("xqT", [HIDDEN, SQ], F16, kind="ExternalInput")
    xkvT = nc.dram_tensor("xkvT", [HIDDEN, SKV], F16, kind="ExternalInput")
    WqT = nc.dram_tensor("WqT", [HIDDEN, HL], F16, kind="ExternalInput")
    WkT = nc.dram_tensor("WkT", [HIDDEN, HL], F16, kind="ExternalInput")
    WvT = nc.dram_tensor("WvT", [HIDDEN, HL], F16, kind="ExternalInput")
    WoT = nc.dram_tensor("WoT", [HL, HIDDEN], F16, kind="ExternalInput")
    onesA = nc.dram_tensor("onesA", [1, 128], F32R, kind="ExternalInput")
    onesB = nc.dram_tensor("onesB", [1, 128], F32R, kind="ExternalInput")
    out_d = nc.dram_tensor("out", [SQ, HIDDEN], F32, kind="ExternalOutput")

    with tc.tile_pool(name="persist", bufs=1) as pp:
        qT = pp.tile([128, 4, SQ], F16)          # 16 KB/part
        onesA_t = pp.tile([1, 128], F32R)
        onesB_t = pp.tile([1, 128], F32R)
        kT = pp.tile([128, 4, SKV], F16)         # 32 KB
        va = pp.tile([128, NYC, NHL, 65], F16)   # 33.3 KB  (y-chunk, head, dv+1)
        xTu = pp.tile([128, 4, SQ], F16)         # 16 KB, natural hd order
        nc.sync.dma_start(out=onesA_t, in_=onesA[:, :])
        nc.sync.dma_start(out=onesB_t, in_=onesB[:, :])

        with tc.tile_pool(name="psProj", bufs=4, space="PSUM") as psProj:
            # ---- Phase 1: q projection  qT[hd, s] = sum_i Wq[hd, i] xq[s, i]
            with tc.tile_pool(name="qin", bufs=1) as qin:
                xqT_t = qin.tile([128, KCH, SQ], F16)
                WqT_t = qin.tile([128, KCH, HL], F16)
                nc.sync.dma_start(
                    out=WqT_t, in_=WqT.rearrange("(k p) n -> p k n", p=128)
                )
                for k in range(KCH):
                    nc.sync.dma_start(
                        out=xqT_t[:, k, :], in_=xqT[k * 128 : (k + 1) * 128, :]
                    )
                # k outermost: compute starts as soon as the first chunks land
                for s in range(2):
                    pts = [psProj.tile([128, 512], F32, tag="proj", name="ptq") for _ in range(4)]
                    for k in range(KCH):
                        for m in range(4):
                            nc.tensor.matmul(
                                pts[m][:, :],
                                lhsT=WqT_t[:, k, m * 128 : (m + 1) * 128],
                                rhs=xqT_t[:, k, s * 512 : (s + 1) * 512],
                                start=(k == 0),
                                stop=(k == KCH - 1),
                            )
                    for m in range(4):
                        nc.vector.tensor_copy(
                            out=qT[:, m, s * 512 : (s + 1) * 512], in_=pts[m][:, :]
                        )

            # ---- Phase 2: k and v projections from xkvT
            with tc.tile_pool(name="kvx", bufs=1) as kvx:
                xkvT_t = kvx.tile([128, KCH, SKV], F16)
                for k in range(KCH):
                    nc.sync.dma_start(
                        out=xkvT_t[:, k, :], in_=xkvT[k * 128 : (k + 1) * 128, :]
                    )
                with tc.tile_pool(name="kvw1", bufs=1) as kvw1:
                    WkT_t = kvw1.tile([128, KCH, HL], F16)
                    nc.sync.dma_start(
                        out=WkT_t, in_=WkT.rearrange("(k p) n -> p k n", p=128)
                    )
                    # kT[hd, y], k outermost within each y-group
                    for yg in range(SKV // 512):
                        pts = [
                            psProj.tile([128, 512], F32, tag="proj", name="ptk")
                            for _ in range(4)
                        ]
                        for k in range(KCH):
                            for m in range(4):
                                nc.tensor.matmul(
                                    pts[m][:, :],
                                    lhsT=WkT_t[:, k, m * 128 : (m + 1) * 128],
                                    rhs=xkvT_t[:, k, yg * 512 : (yg + 1) * 512],
                                    start=(k == 0),
                                    stop=(k == KCH - 1),
                                )
                        for m in range(4):
                            nc.vector.tensor_copy(
                                out=kT[:, m, yg * 512 : (yg + 1) * 512],
                                in_=pts[m][:, :],
                            )
                with tc.tile_pool(name="kvw2", bufs=1) as kvw2:
                    WvT_t = kvw2.tile([128, KCH, HL], F16)
                    nc.sync.dma_start(
                        out=WvT_t, in_=WvT.rearrange("(k p) n -> p k n", p=128)
                    )
                    # v_aug[y, h, 0:64] in two half-width passes (heads 0-3,
                    # then 4-7) so early attention chains unblock sooner
                    for half in range(2):
                        hlo = half * 4
                        for yc in range(NYC):
                            pt = psProj.tile([128, 512], F32, tag="proj")
                            for k in range(KCH):
                                nc.tensor.matmul(
                                    pt[:, 0:256],
                                    lhsT=xkvT_t[:, k, yc * 128 : (yc + 1) * 128],
                                    rhs=WvT_t[:, k, hlo * 64 : (hlo + 4) * 64],
                                    start=(k == 0),
                                    stop=(k == KCH - 1),
                                )
                            nc.vector.tensor_copy(
                                out=va[:, yc, hlo : hlo + 4, 0:64],
                                in_=pt[:, 0:256].rearrange("p (h d) -> p h d", h=4),
                            )
                    nc.vector.memset(va[:, :, :, 64:65], 1.0)

        # ---- Phase 3: attention + normalization + out projection
        with (
            tc.tile_pool(name="attp", bufs=1) as attp,
            tc.tile_pool(name="zrowp", bufs=4) as zrowp,
            tc.tile_pool(name="rzp", bufs=2) as rzp,
            tc.tile_pool(name="outw", bufs=1) as outw,
            tc.tile_pool(name="outstage", bufs=3) as outstage,
            tc.tile_pool(name="psS2", bufs=1, space="PSUM") as psS2,
            tc.tile_pool(name="psMisc", bufs=1, space="PSUM") as psMisc,
        ):
            WoT_t = outw.tile([128, 4, HIDDEN], F16)
            nc.sync.dma_start(
                out=WoT_t, in_=WoT.rearrange("(j p) n -> p j n", p=128)
            )

            NYB = NYC // 2  # 8 double-y blocks per chain
            LAG = 3         # attv trails scores/exp by this many blocks

            _scnt = [0]

            def emit_scores(h, xc, yb):
                pq = (h % 2) * 64
                m = h // 2
                _scnt[0] += 1
                pscr = psS2.tile(
                    [128, 2, 512], F32, tag=f"scores{_scnt[0] % 3}", name="pscr"
                )
                last = None
                for i in range(2):
                    yc = 2 * yb + i
                    last = nc.tensor.matmul(
                        pscr[:, i, :],
                        lhsT=kT[pq : pq + 64, m, yc * 128 : (yc + 1) * 128],
                        rhs=qT[pq : pq + 64, m, xc * 512 : (xc + 1) * 512],
                        start=True,
                        stop=True,
                    )
                at = attp.tile(
                    [128, 2, 512], F16, tag=f"att{_scnt[0] % 10}", name="at"
                )
                nc.scalar.activation(
                    out=at[:, :, :].rearrange("p a b -> p (a b)"),
                    in_=pscr[:, :, :].rearrange("p a b -> p (a b)"),
                    func=mybir.ActivationFunctionType.Exp,
                    scale=SCALE,
                )
                return at, last

            def emit_attv(h, po, at, yb, order_after=None):
                for i in range(2):
                    yc = 2 * yb + i
                    mm = nc.tensor.matmul(
                        po[:, :],
                        lhsT=va[:, yc, h, :],
                        rhs=at[:, i, :],
                        start=(yc == 0),
                        stop=(yc == NYC - 1),
                    )
                    if i == 0 and order_after is not None:
                        # force the PE static order to keep attv trailing the
                        # scores stream by LAG blocks (hides the ACT latency)
                        add_dep_helper(
                            mm.ins,
                            order_after.ins,
                            sync=False,
                            reason="attv trails scores pipeline",
                        )

            for xc in range(SQ // 512):
                for j in range(4):
                    hA, hB = 2 * j, 2 * j + 1
                    poA = psMisc.tile([65, 512], F32, tag="attv", bufs=1, name="poA")
                    poB = psMisc.tile([65, 512], F32, tag="attv2", bufs=1, name="poB")
                    attsA, attsB = [], []
                    # two independent chains interleaved: while one waits on
                    # ACT, the PE works the other
                    scoreMM = []
                    for yb in range(NYB):
                        atA, _ = emit_scores(hA, xc, yb)
                        atB, lastB = emit_scores(hB, xc, yb)
                        attsA.append(atA)
                        attsB.append(atB)
                        scoreMM.append(lastB)
                        if yb >= LAG:
                            emit_attv(hA, poA, attsA[yb - LAG], yb - LAG,
                                      order_after=scoreMM[yb])
                            emit_attv(hB, poB, attsB[yb - LAG], yb - LAG)
                    for yb in range(NYB - LAG, NYB):
                        emit_attv(hA, poA, attsA[yb], yb)
                        emit_attv(hB, poB, attsB[yb], yb)

                    zrs = []
                    for h, po in ((hA, poA), (hB, poB)):
                        ps_off = (h % 2) * 64
                        nc.vector.tensor_copy(
                            out=xTu[
                                ps_off : ps_off + 64, j, xc * 512 : (xc + 1) * 512
                            ],
                            in_=po[0:64, :],
                        )
                        zr = zrowp.tile([1, 512], F32R, tag="zrow", name="zr")
                        nc.vector.tensor_copy(out=zr[0:1, :], in_=po[64:65, :])
                        zrs.append(zr)
                    # normalize chunk j (bcast psum shares the scores slots)
                    pb = psS2.tile([128, 512], F32, tag="scores0", name="pb")
                    nc.tensor.matmul(
                        pb[:, :], lhsT=onesA_t[0:1, :], rhs=zrs[0][0:1, :],
                        start=True, stop=False,
                    )
                    nc.tensor.matmul(
                        pb[:, :], lhsT=onesB_t[0:1, :], rhs=zrs[1][0:1, :],
                        start=False, stop=True,
                    )
                    rz = rzp.tile([128, 512], F32, tag="rz", name="rz")
                    nc.vector.reciprocal(out=rz[:, :], in_=pb[:, :])
                    nc.vector.tensor_mul(
                        xTu[:, j, xc * 512 : (xc + 1) * 512],
                        xTu[:, j, xc * 512 : (xc + 1) * 512],
                        rz[:, :],
                    )

            # out projection: out[s, o] = sum_j xTu[:, j, s].T @ WoT[:, j, o]
            for sc in range(SQ // 128):
                for oc in range(HIDDEN // 512):
                    pt = psS2.tile([128, 512], F32, tag="scores1", name="pto")
                    for j in range(4):
                        nc.tensor.matmul(
                            pt[:, :],
                            lhsT=xTu[:, j, sc * 128 : (sc + 1) * 128],
                            rhs=WoT_t[:, j, oc * 512 : (oc + 1) * 512],
                            start=(j == 0),
                            stop=(j == 3),
                        )
                    ot = outstage.tile([128, 512], F32, tag="out")
                    nc.vector.tensor_copy(out=ot[:, :], in_=pt[:, :])
                    nc.sync.dma_start(
                        out=out_d[sc * 128 : (sc + 1) * 128, oc * 512 : (oc + 1) * 512],
                        in_=ot[:, :],
                    )


_NC = None


def _get_nc():
    global _NC
    if _NC is None:
        nc = bass.Bass(trn_type="TRN2")
        with tile.TileContext(nc) as tc:
            _emit(nc, tc)
        _legalize_waits(nc)
        _NC = nc
    return _NC


def _prep_inputs(xq, xkv, Wq, Wkv, Wout):
    xq = np.asarray(xq, dtype=np.float16)
    xkv = np.asarray(xkv, dtype=np.float16)
    Wq = np.asarray(Wq, dtype=np.float16)
    Wkv = np.asarray(Wkv, dtype=np.float16)
    Wout = np.asarray(Wout, dtype=np.float16)

    onesA = np.zeros((1, 128), np.float32)
    onesA[0, 0:64] = 1.0
    onesB = np.zeros((1, 128), np.float32)
    onesB[0, 64:128] = 1.0

    xqT = [np.ascontiguousarray(xq[b].T) for b in range(B)]
    xkvT = [np.ascontiguousarray(xkv[b].T) for b in range(B)]

    per_hg = []
    for hg in range(2):
        hs = slice(hg * HL, (hg + 1) * HL)
        WqTh = np.ascontiguousarray(Wq[hs].T)
        WkTh = np.ascontiguousarray(Wkv[hs].T)
        WvTh = np.ascontiguousarray(Wkv[HIDDEN + hg * HL : HIDDEN + (hg + 1) * HL].T)
        WoTh = np.ascontiguousarray(Wout[:, hs].T)
        per_hg.append((WqTh, WkTh, WvTh, WoTh))

    in_maps = []
    for c in range(NCORES):
        b, hg = c // 2, c % 2
        WqTh, WkTh, WvTh, WoTh = per_hg[hg]
        in_maps.append(
            {
                "xqT": xqT[b],
                "xkvT": xkvT[b],
                "WqT": WqTh,
                "WkT": WkTh,
                "WvT": WvTh,
                "WoT": WoTh,
                "onesA": onesA,
                "onesB": onesB,
            }
        )
    return in_maps


def run_sharded(xq, xkv, Wq, Wkv, Wout, bout, trace=False, **kwargs):
    """Build+run the SPMD kernel; returns (full_output, BassKernelResults)."""
    nc = _get_nc()
    in_maps = _prep_inputs(xq, xkv, Wq, Wkv, Wout)
    res = run_bass_kernel_spmd(
        nc, in_maps, core_ids=list(range(NCORES)), trace=trace, **kwargs
    )
    bout = np.asarray(bout, dtype=np.float32)
    out = np.empty((B, SQ, HIDDEN), np.float32)
    for b in range(B):
        out[b] = res.results[2 * b]["out"] + res.results[2 * b + 1]["out"]
    out += bout[None, None, :]
    return out, res


def kernel(xq, xkv, Wq, Wkv, Wout, bout):
    out, _ = run_sharded(xq, xkv, Wq, Wkv, Wout, bout)
    return out



# revision 7
# speedup vs baseline: 1.3925x; 1.1882x over previous
"""Cross-attention block (nn_CABlock) on 8 TRN2 NeuronCores.

Reference (per batch b):
    q  = xq @ Wq.T            -> [SQ, H]   split heads [SQ, 16, 64]
    kv = xkv @ Wkv.T          -> [SKV, 2H] split [SKV, 2, 16, 64]
    att = softmax(q k^T / sqrt(64))
    x   = att @ v  (merge heads)
    out = x @ Wout.T + bout
Sharding: 8 cores = 4 batches x 2 head-groups (8 heads each).  Host sums
the two partial out-projections per batch and adds bout.

All matmul operands are fp16 (1 cycle/row on the PE at 2.4 GHz; fp32
accumulation in PSUM, so precision is set by input quantization only,
~3e-4).  The PE instruction stream is emitted fully interleaved: the
q/k/v projections, out-projection and Z-broadcasts are chopped into
8-matmul "groups" and scheduled as FILLER inside the attention chains,
so the PE never idles waiting on the ACT engine's Exp stream (idle gaps
re-throttle the PE to 1.2 GHz via HAM; density keeps it at 2.4 GHz).

Layout (per core):
  - host pre-transposes: xqT [H, SQ], xkvT [H, SKV]; weights column-sliced.
  - qT/kT: [128 hd, m-chunk, s|y] in natural head order (head h -> chunk
    h//2, partition offset (h%2)*64).
  - va [128 y, yc, head, 65]: v columns + a ones column -> att @ va also
    emits the softmax denominator Z as row 64.  No max subtraction
    (logits are O(1), exp cannot overflow fp16).
  - scores computed transposed, sT[y, x], 2 y-chunks per PSUM tile; one
    W=1024 Exp (fused 1/8 scale) per tile -> fp16 att weights.
  - xTu [512 hd, SQ] collects unnormalized head outputs; Z rows are
    reciprocal'd on DVE ([2,512] per chain), broadcast to 128 partitions
    with K=1 ones-matmuls into PSUM, and multiplied in once per chain.
  - out-projection contracts xTu chunks with WoutT slices, streamed to
    DRAM as [128, 512] fp32 tiles.

PSUM budget (8 banks): scores 2 tags x 2 banks + attv poA/poB 1+1 +
misc (projections / broadcasts) 2 rotating.
"""

import sys

sys.path.insert(0, "/opt/trn_rl_repo")

import numpy as np

import concourse.bass as bass
import concourse.mybir as mybir
import concourse.tile as tile
from concourse.bass_utils import run_bass_kernel_spmd
from concourse.tile import add_dep_helper

F32 = mybir.dt.float32
F32R = mybir.dt.float32r
F16 = mybir.dt.float16

HIDDEN = 1024
NUM_HEADS = 16
HEAD_DIM = 64
B = 4
SQ = 1024
SKV = 2048
NCORES = 8
NHL = 8          # heads per core
HL = NHL * HEAD_DIM  # 512, local hd width
SCALE = HEAD_DIM ** -0.5
KCH = HIDDEN // 128  # 8 contraction chunks for the projections
NYC = SKV // 128     # 16 key/value row chunks
NYB = NYC // 2       # 8 double-y blocks per attention chain
LAG = 3              # attv trails scores/exp by this many blocks


def _legalize_waits(nc, limit=1):
    """The walrus build in this container accepts only ~1 sync-wait per
    instruction struct; spill excess waits onto preceding engine NoOps."""
    import bass_rust

    ctr = 0
    for fn in nc.m.functions:
        for blk in fn.blocks:
            out = []
            changed = False
            for inst in blk.instructions:
                si = inst.sync_info
                ws = list(si.on_wait) if si is not None and si.on_wait else []
                if len(ws) > limit:
                    spill, keep = ws[:-limit], ws[-limit:]
                    for w in spill:
                        ctr += 1
                        nop = mybir.InstNoOp(name=f"ant-waitnop-{ctr}", ins=[], outs=[])
                        nop.engine = inst.engine
                        nop.sync_info = bass_rust.SyncInfo(on_wait=[w], on_update=[])
                        out.append(nop)
                    si.on_wait = keep
                    changed = True
                out.append(inst)
            if changed:
                blk.instructions = out
    return ctr


def _emit(nc, tc):
    xqT = nc.dram_tensor("xqT", [HIDDEN, SQ], F16, kind="ExternalInput")
    xkvT = nc.dram_tensor("xkvT", [HIDDEN, SKV], F16, kind="ExternalInput")
    WqT = nc.dram_tensor("WqT", [HIDDEN, HL], F16, kind="ExternalInput")
    WkT = nc.dram_tensor("WkT", [HIDDEN, HL], F16, kind="ExternalInput")
    WvT = nc.dram_tensor("WvT", [HIDDEN, HL], F16, kind="ExternalInput")
    WoT = nc.dram_tensor("WoT", [HL, HIDDEN], F16, kind="ExternalInput")
    onesA = nc.dram_tensor("onesA", [1, 128], F16, kind="ExternalInput")
    onesB = nc.dram_tensor("onesB", [1, 128], F16, kind="ExternalInput")
    out_d = nc.dram_tensor("out", [SQ, HIDDEN], F32, kind="ExternalOutput")

    with tc.tile_pool(name="persist", bufs=1) as pp:
        xqT_t = pp.tile([128, KCH, SQ], F16)       # 16 KB/part
        xkvT_t = pp.tile([128, KCH, SKV], F16)     # 32 KB
        WqT_t = pp.tile([128, KCH, HL], F16)       # 8 KB
        WkT_t = pp.tile([128, KCH, HL], F16)       # 8 KB
        WvT_t = pp.tile([128, KCH, HL], F16)       # 8 KB
        WoT_t = pp.tile([128, 4, HIDDEN], F16)     # 8 KB
        qT = pp.tile([128, 4, SQ], F16)            # 8 KB
        kT = pp.tile([128, 4, SKV], F16)           # 16 KB
        va = pp.tile([128, NYC, NHL, 65], F16)     # 16.6 KB
        xTu = pp.tile([128, 4, SQ], F16)           # 8 KB
        onesA_t = pp.tile([1, 128], F16)
        onesB_t = pp.tile([1, 128], F16)

        # DMA issue order = earliest-consumer-first; the k/q projections
        # are emitted k-outer so compute starts as chunks land.
        nc.sync.dma_start(out=WkT_t, in_=WkT.rearrange("(k p) n -> p k n", p=128))
        nc.sync.dma_start(out=WqT_t, in_=WqT.rearrange("(k p) n -> p k n", p=128))
        nc.sync.dma_start(out=onesA_t, in_=onesA[:, :])
        nc.sync.dma_start(out=onesB_t, in_=onesB[:, :])
        for k in range(KCH):
            nc.sync.dma_start(
                out=xkvT_t[:, k, :], in_=xkvT[k * 128 : (k + 1) * 128, :]
            )
            nc.sync.dma_start(
                out=xqT_t[:, k, :], in_=xqT[k * 128 : (k + 1) * 128, :]
            )
        nc.sync.dma_start(out=WvT_t, in_=WvT.rearrange("(k p) n -> p k n", p=128))
        nc.sync.dma_start(out=WoT_t, in_=WoT.rearrange("(j p) n -> p j n", p=128))

        with (
            tc.tile_pool(name="psS", bufs=1, space="PSUM") as psS,
            tc.tile_pool(name="psA", bufs=1, space="PSUM") as psA,
            tc.tile_pool(name="psM", bufs=1, space="PSUM") as psM,
            tc.tile_pool(name="attp", bufs=1) as attp,
            tc.tile_pool(name="zrp", bufs=2) as zrp,
            tc.tile_pool(name="rzp", bufs=2) as rzp,
            tc.tile_pool(name="outst", bufs=3) as outst,
        ):
            nc.vector.memset(va[:, :, :, 64:65], 1.0)

            _mcnt = [0]

            def misc_tile():
                _mcnt[0] += 1
                return psM.tile(
                    [128, 512], F32, tag=f"m{_mcnt[0] % 2}", name="pm"
                )

            # ---- filler group emitters (8 matmuls + a DVE copy each) ----
            def g_qproj(m, s):
                pt = misc_tile()
                for k in range(KCH):
                    nc.tensor.matmul(
                        pt[:, :],
                        lhsT=WqT_t[:, k, m * 128 : (m + 1) * 128],
                        rhs=xqT_t[:, k, s * 512 : (s + 1) * 512],
                        start=(k == 0),
                        stop=(k == KCH - 1),
                    )
                nc.vector.tensor_copy(
                    out=qT[:, m, s * 512 : (s + 1) * 512], in_=pt[:, :]
                )

            def g_kproj(m, yg):
                pt = misc_tile()
                for k in range(KCH):
                    nc.tensor.matmul(
                        pt[:, :],
                        lhsT=WkT_t[:, k, m * 128 : (m + 1) * 128],
                        rhs=xkvT_t[:, k, yg * 512 : (yg + 1) * 512],
                        start=(k == 0),
                        stop=(k == KCH - 1),
                    )
                nc.vector.tensor_copy(
                    out=kT[:, m, yg * 512 : (yg + 1) * 512], in_=pt[:, :]
                )

            def g_vproj(half, yc):
                pt = misc_tile()
                hlo = half * 4
                for k in range(KCH):
                    nc.tensor.matmul(
                        pt[:, 0:256],
                        lhsT=xkvT_t[:, k, yc * 128 : (yc + 1) * 128],
                        rhs=WvT_t[:, k, hlo * 64 : (hlo + 4) * 64],
                        start=(k == 0),
                        stop=(k == KCH - 1),
                    )
                nc.vector.tensor_copy(
                    out=va[:, yc, hlo : hlo + 4, 0:64],
                    in_=pt[:, 0:256].rearrange("p (h d) -> p h d", h=4),
                )

            def g_outproj(sc, oc):
                pt = misc_tile()
                for j in range(4):
                    nc.tensor.matmul(
                        pt[:, :],
                        lhsT=xTu[:, j, sc * 128 : (sc + 1) * 128],
                        rhs=WoT_t[:, j, oc * 512 : (oc + 1) * 512],
                        start=(j == 0),
                        stop=(j == 3),
                    )
                ot = outst.tile([128, 512], F32, tag="out")
                nc.vector.tensor_copy(out=ot[:, :], in_=pt[:, :])
                nc.sync.dma_start(
                    out=out_d[
                        sc * 128 : (sc + 1) * 128, oc * 512 : (oc + 1) * 512
                    ],
                    in_=ot[:, :],
                )

            # ---- attention chain emitters ----
            _scnt = [0]

            def emit_scores(h, xc, yb):
                pq = (h % 2) * 64
                m = h // 2
                _scnt[0] += 1
                pscr = psS.tile(
                    [128, 2, 512], F32, tag=f"s{_scnt[0] % 2}", name="pscr"
                )
                last = None
                for i in range(2):
                    yc = 2 * yb + i
                    last = nc.tensor.matmul(
                        pscr[:, i, :],
                        lhsT=kT[pq : pq + 64, m, yc * 128 : (yc + 1) * 128],
                        rhs=qT[pq : pq + 64, m, xc * 512 : (xc + 1) * 512],
                        start=True,
                        stop=True,
                    )
                at = attp.tile(
                    [128, 2, 512], F16, tag=f"att{_scnt[0] % 10}", name="at"
                )
                nc.scalar.activation(
                    out=at[:, :, :].rearrange("p a b -> p (a b)"),
                    in_=pscr[:, :, :].rearrange("p a b -> p (a b)"),
                    func=mybir.ActivationFunctionType.Exp,
                    scale=SCALE,
                )
                return at, last

            def emit_attv(h, po, at, yb, order_after=None):
                for i in range(2):
                    yc = 2 * yb + i
                    mm = nc.tensor.matmul(
                        po[:, :],
                        lhsT=va[:, yc, h, :],
                        rhs=at[:, i, :],
                        start=(yc == 0),
                        stop=(yc == NYC - 1),
                    )
                    if i == 0 and order_after is not None:
                        # static-order hint: keep attv trailing the scores
                        # stream (hides the Exp latency)
                        add_dep_helper(
                            mm.ins,
                            order_after.ins,
                            sync=False,
                            reason="attv trails scores pipeline",
                        )

            def run_chain(xc, j, fillers):
                hA, hB = 2 * j, 2 * j + 1
                poA = psA.tile([65, 512], F32, tag="a0", bufs=1, name="poA")
                poB = psA.tile([65, 512], F32, tag="a1", bufs=1, name="poB")
                # distribute filler groups over the 8 yb slots, front-loaded
                per_slot = [[] for _ in range(NYB)]
                q, r = divmod(len(fillers), NYB)
                pos = 0
                for sl in range(NYB):
                    take = q + (1 if sl < r else 0)
                    per_slot[sl] = fillers[pos : pos + take]
                    pos += take
                attsA, attsB, scoreMM = [], [], []
                for yb in range(NYB):
                    atA, _ = emit_scores(hA, xc, yb)
                    atB, lastB = emit_scores(hB, xc, yb)
                    attsA.append(atA)
                    attsB.append(atB)
                    scoreMM.append(lastB)
                    for f in per_slot[yb]:
                        f()
                    if yb >= LAG:
                        emit_attv(hA, poA, attsA[yb - LAG], yb - LAG,
                                  order_after=scoreMM[yb])
                        emit_attv(hB, poB, attsB[yb - LAG], yb - LAG)
                for yb in range(NYB - LAG, NYB):
                    emit_attv(hA, poA, attsA[yb], yb)
                    emit_attv(hB, poB, attsB[yb], yb)

                # normalization: Z rows -> broadcast to 128 partitions via
                # K=1 ones-matmuls (PSUM), copy out fast, reciprocal on
                # SBUF (PSUM bank freed after ~0.6us), multiply once.
                zrA = zrp.tile([1, 512], F16, tag="zA", name="zrA")
                zrB = zrp.tile([1, 512], F16, tag="zB", name="zrB")
                nc.vector.tensor_copy(out=zrA[0:1, :], in_=poA[64:65, :])
                nc.vector.tensor_copy(out=zrB[0:1, :], in_=poB[64:65, :])
                for h, po in ((hA, poA), (hB, poB)):
                    ps_off = (h % 2) * 64
                    nc.vector.tensor_copy(
                        out=xTu[
                            ps_off : ps_off + 64, j, xc * 512 : (xc + 1) * 512
                        ],
                        in_=po[0:64, :],
                    )
                pb = misc_tile()
                nc.tensor.matmul(
                    pb[:, :], lhsT=onesA_t[0:1, :], rhs=zrA[0:1, :],
                    start=True, stop=False,
                )
                nc.tensor.matmul(
                    pb[:, :], lhsT=onesB_t[0:1, :], rhs=zrB[0:1, :],
                    start=False, stop=True,
                )
                pbs = rzp.tile([128, 512], F32, tag="pbs", name="pbs")
                nc.vector.tensor_copy(out=pbs[:, :], in_=pb[:, :])
                rz = rzp.tile([128, 512], F16, tag="rz", name="rz")
                with nc.allow_low_precision(reason="1/Z in fp16, ~5e-4 rel"):
                    nc.vector.reciprocal(out=rz[:, :], in_=pbs[:, :])
                nc.vector.tensor_mul(
                    xTu[:, j, xc * 512 : (xc + 1) * 512],
                    xTu[:, j, xc * 512 : (xc + 1) * 512],
                    rz[:, :],
                )

            # ---- schedule: prefix groups, then chains with filler ----
            for yg in range(2):
                g_kproj(0, yg)
            g_qproj(0, 0)
            for yg in range(2, 4):
                g_kproj(0, yg)
            for yc in range(4):
                g_vproj(0, yc)

            sched = {
                0: (
                    [lambda yc=yc: g_vproj(0, yc) for yc in range(4, 8)]
                    + [lambda: g_qproj(0, 1)]
                    + [lambda yc=yc: g_vproj(0, yc) for yc in range(8, 12)]
                    + [lambda yg=yg: g_kproj(1, yg) for yg in range(2)]
                    + [lambda yc=yc: g_vproj(0, yc) for yc in range(12, 16)]
                    + [lambda yg=yg: g_kproj(1, yg) for yg in range(2, 4)]
                    + [lambda: g_qproj(1, 0)]
                ),
                1: (
                    [lambda yc=yc: g_vproj(1, yc) for yc in range(0, 4)]
                    + [lambda: g_qproj(1, 1)]
                    + [lambda yc=yc: g_vproj(1, yc) for yc in range(4, 8)]
                    + [lambda yg=yg: g_kproj(2, yg) for yg in range(4)]
                    + [lambda: g_qproj(2, 0)]
                ),
                2: (
                    [lambda yc=yc: g_vproj(1, yc) for yc in range(8, 12)]
                    + [lambda yg=yg: g_kproj(3, yg) for yg in range(2)]
                    + [lambda yc=yc: g_vproj(1, yc) for yc in range(12, 16)]
                    + [lambda yg=yg: g_kproj(3, yg) for yg in range(2, 4)]
                    + [lambda: g_qproj(2, 1), lambda: g_qproj(3, 0)]
                ),
                3: [lambda: g_qproj(3, 1)],
                4: [
                    lambda sc=sc, oc=oc: g_outproj(sc, oc)
                    for sc in range(2)
                    for oc in range(2)
                ],
                5: [
                    lambda sc=sc, oc=oc: g_outproj(sc, oc)
                    for sc in range(2, 4)
                    for oc in range(2)
                ],
                6: [],
                7: [],
            }

            for c in range(8):
                xc, j = c // 4, c % 4
                run_chain(xc, j, sched[c])

            for sc in range(4, 8):
                for oc in range(2):
                    g_outproj(sc, oc)


_NC = None


def _get_nc():
    global _NC
    if _NC is None:
        nc = bass.Bass(trn_type="TRN2")
        with tile.TileContext(nc) as tc:
            _emit(nc, tc)
        _legalize_waits(nc)
        _NC = nc
    return _NC


def _prep_inputs(xq, xkv, Wq, Wkv, Wout):
    xq = np.asarray(xq, dtype=np.float16)
    xkv = np.asarray(xkv, dtype=np.float16)
    Wq = np.asarray(Wq, dtype=np.float16)
    Wkv = np.asarray(Wkv, dtype=np.float16)
    Wout = np.asarray(Wout, dtype=np.float16)

    onesA = np.zeros((1, 128), np.float16)
    onesA[0, 0:64] = 1.0
    onesB = np.zeros((1, 128), np.float16)
    onesB[0, 64:128] = 1.0

    xqT = [np.ascontiguousarray(xq[b].T) for b in range(B)]
    xkvT = [np.ascontiguousarray(xkv[b].T) for b in range(B)]

    per_hg = []
    for hg in range(2):
        hs = slice(hg * HL, (hg + 1) * HL)
        WqTh = np.ascontiguousarray(Wq[hs].T)
        WkTh = np.ascontiguousarray(Wkv[hs].T)
        WvTh = np.ascontiguousarray(Wkv[HIDDEN + hg * HL : HIDDEN + (hg + 1) * HL].T)
        WoTh = np.ascontiguousarray(Wout[:, hs].T)
        per_hg.append((WqTh, WkTh, WvTh, WoTh))

    in_maps = []
    for c in range(NCORES):
        b, hg = c // 2, c % 2
        WqTh, WkTh, WvTh, WoTh = per_hg[hg]
        in_maps.append(
            {
                "xqT": xqT[b],
                "xkvT": xkvT[b],
                "WqT": WqTh,
                "WkT": WkTh,
                "WvT": WvTh,
                "WoT": WoTh,
                "onesA": onesA,
                "onesB": onesB,
            }
        )
    return in_maps


def run_sharded(xq, xkv, Wq, Wkv, Wout, bout, trace=False, **kwargs):
    """Build+run the SPMD kernel; returns (full_output, BassKernelResults)."""
    nc = _get_nc()
    in_maps = _prep_inputs(xq, xkv, Wq, Wkv, Wout)
    res = run_bass_kernel_spmd(
        nc, in_maps, core_ids=list(range(NCORES)), trace=trace, **kwargs
    )
    bout = np.asarray(bout, dtype=np.float32)
    out = np.empty((B, SQ, HIDDEN), np.float32)
    for b in range(B):
        out[b] = res.results[2 * b]["out"] + res.results[2 * b + 1]["out"]
    out += bout[None, None, :]
    return out, res


def kernel(xq, xkv, Wq, Wkv, Wout, bout):
    out, _ = run_sharded(xq, xkv, Wq, Wkv, Wout, bout)
    return out
